# revision 2
# baseline (speedup 1.0000x reference)
"""Trainium2 Bass kernel for nn_Decoder1 (CVRP pointer-network greedy decoder).

Self-contained: builds one SPMD Bass program (8 cores, batch-parallel:
1024 rows -> 128 per core), runs it via run_bass_kernel_spmd, and
reassembles full outputs (actions int32 [1024,142], logps f32 [1024]).

Per-core design (B=128 batch rows live on the 128 SBUF partitions for all
decode bookkeeping; the three per-row attention contractions run as per-row
PE matmuls against SBUF-resident transposed encodings):

  encT [d, (b,n)]  = enc^T                (logits stationary + k/v source)
  kT   [dout,(b,n)] = (enc@attn_k)^T      (compat stationary)
  v_nT [n, (b,dout)] = enc@attn_v         (out-contraction stationary)

Step: dec_inT/qT dense matmuls -> per-b compat matmuls into PSUM [n,(b,h)]
with the -1e9 mask folded in via one mask @ (-1e9*I kron 1_8) accumulate
matmul -> exp on ACT (no max-subtract needed; masked lanes underflow to 0)
-> 8 ones-matmuls give softmax denominators -> per-b out-contraction
(cross-head [dout,(b,h')] in PSUM, diagonal extracted + 1/S scaled with
mask-multiply + reduce) -> u = (attn_fc@prob_k^T)^T @ outT -> per-b logits
matmuls [n,b] -> PE transpose -> tanh/mask -> argmax via DVE max/max_index
-> mask/dyn/logp bookkeeping -> next state via indirect-DMA row gather.
All ln() calls deferred to one batched ACT call after the loop (keeps the
ACT table set fixed at exp/tanh in-loop); logp = -sum_t ln(W_t).
"""

import os
import sys
import threading
from contextlib import ExitStack

import numpy as np

for _p in ("/opt/trn_rl_repo", "/root/.axon_site/_ro/trn_rl_repo"):
    if os.path.isdir(_p) and _p not in sys.path:
        sys.path.insert(0, _p)

import concourse.bass as bass
import concourse.mybir as mybir
from concourse import bacc
from concourse.tile import TileContext

F32 = mybir.dt.float32
I32 = mybir.dt.int32
U32 = mybir.dt.uint32
OP = mybir.AluOpType
AF = mybir.ActivationFunctionType

B = 128      # batch rows per core
N = 101      # nodes
D = 128      # model dim
H = 8        # heads
HD = D // H
INV_HD = float(1.0 / np.sqrt(HD))
INV_D = float(1.0 / np.sqrt(D))
N_CORES = 8

_cache = {}
_cache_lock = threading.Lock()


def _build(n_steps: int, unrolled: bool = False):
    nc = bacc.Bacc("TRN2", target_bir_lowering=False, debug=False,
                   num_devices=N_CORES)

    enc_rows = nc.dram_tensor("enc_rows", [B * N, D], F32, kind="ExternalInput")
    demand_d = nc.dram_tensor("demand", [B, N], F32, kind="ExternalInput")
    cap_d = nc.dram_tensor("capcity", [B, 1], F32, kind="ExternalInput")
    capfull_d = nc.dram_tensor("capfull", [B, 1], F32, kind="ExternalInput")
    pool_d = nc.dram_tensor("pool", [B, D], F32, kind="ExternalInput")
    fcw_main_d = nc.dram_tensor("fc_w_main", [D, D], F32, kind="ExternalInput")
    fcw_last_d = nc.dram_tensor("fc_w_last", [1, D], F32, kind="ExternalInput")
    fc1_d = nc.dram_tensor("fc1_w", [D, D], F32, kind="ExternalInput")
    attn_w_d = nc.dram_tensor("attn_w", [D, D], F32, kind="ExternalInput")
    attn_k_d = nc.dram_tensor("attn_k", [D, D], F32, kind="ExternalInput")
    attn_v_d = nc.dram_tensor("attn_v", [D, D], F32, kind="ExternalInput")
    c2_d = nc.dram_tensor("c2", [D, D], F32, kind="ExternalInput")
    rconst_d = nc.dram_tensor("r_const", [H, B], F32, kind="ExternalInput")
    m8_d = nc.dram_tensor("m8", [B, H], F32, kind="ExternalInput")
    ident_d = nc.dram_tensor("ident_in", [B, B], F32, kind="ExternalInput")
    iota_d = nc.dram_tensor("iota_in", [B, N], F32, kind="ExternalInput")
    rowbase_d = nc.dram_tensor("rowbase_in", [B, 1], I32, kind="ExternalInput")

    actions_d = nc.dram_tensor("actions", [B, n_steps], I32, kind="ExternalOutput")
    logps_d = nc.dram_tensor("logps", [B, 1], F32, kind="ExternalOutput")

    with TileContext(nc) as tc, ExitStack() as ctx:
        res = ctx.enter_context(tc.tile_pool(name="res", bufs=1))
        stage = ctx.enter_context(tc.tile_pool(name="stage", bufs=3))
        pp = ctx.enter_context(tc.tile_pool(name="pp", bufs=1, space="PSUM"))
        pshared = ctx.enter_context(tc.tile_pool(name="pshared", bufs=2, space="PSUM"))

        encT = res.tile([D, B * N], F32)
        kT = res.tile([D, B * N], F32)
        v_nT = res.tile([B, B * D], F32)
        e_sb = res.tile([B, B * H], F32)
        q_bd = res.tile([D, B * H], F32)
        negI = res.tile([B, B], F32)
        ident = res.tile([B, B], F32)
        fcw_main = res.tile([D, D], F32)
        fcw_last = res.tile([1, D], F32)
        attn_w = res.tile([D, D], F32)
        c2 = res.tile([D, D], F32)
        pooledT = res.tile([D, B], F32)
        demand = res.tile([B, N], F32)
        mask = res.tile([B, N], F32)
        mask1 = res.tile([B, N], F32)
        neg9mask = res.tile([B, N], F32)
        onehot = res.tile([B, N], F32)
        dgt = res.tile([B, N], F32)
        mx = res.tile([B, N], F32)
        scr101 = res.tile([B, N], F32)
        iota_nf = res.tile([B, N], F32)
        rowbase_i = res.tile([B, 1], I32)
        dyn = res.tile([B, 1], F32)
        capf = res.tile([B, 1], F32)
        onesc = res.tile([B, 1], F32)
        R_const = res.tile([H, B], F32)
        M8 = res.tile([B, H], F32)
        stateT = res.tile([D, B], F32)
        state_g = res.tile([B, D], F32)
        dec_inT = res.tile([D, B], F32)
        outT_u = res.tile([D, B], F32)
        outT_s = res.tile([D, B], F32)
        u_sb = res.tile([D, B], F32)
        lt_nb = res.tile([B, B], F32)
        ltanh = res.tile([B, N], F32)
        logits_sb = res.tile([B, N], F32)
        Sinv_sb = res.tile([B, H], F32)
        SinvT_sb = res.tile([H, B], F32)
        max8 = res.tile([B, 8], F32)
        idx8u = res.tile([B, 8], U32)
        idx_f = res.tile([B, 1], F32)
        idx_i = res.tile([B, 1], I32)
        gidx_i = res.tile([B, 1], I32)
        neg_lmax = res.tile([B, 1], F32)
        sume = res.tile([B, 1], F32)
        sm1_old = res.tile([B, 1], F32)
        nd_old = res.tile([B, 1], F32)
        sm1_new = res.tile([B, 1], F32)
        nd_new = res.tile([B, 1], F32)
        sd = res.tile([B, 1], F32)
        gd = res.tile([B, 1], F32)
        gd_i = res.tile([B, 1], I32)
        nd_old_i = res.tile([B, 1], I32)
        dmg = res.tile([B, 1], F32)
        dep = res.tile([B, 1], F32)
        Wsel = res.tile([B, 1], F32)
        W_buf = res.tile([B, n_steps], F32)
        lnW = res.tile([B, n_steps], F32)
        sumln = res.tile([B, 1], F32)
        logp_out = res.tile([B, 1], F32)
        act_buf = res.tile([B, n_steps], I32)
        ps_dyT_sb = res.tile([1, B], F32)

        psc = pp.tile([B, B * H], F32)
        pso = pp.tile([B, B * H], F32)
        ps_rep = pp.tile([D, B], F32)
        ps_dyT = pp.tile([1, B], F32)

        # ============ INIT ============
        nc.gpsimd.dma_start(fcw_main[:], fcw_main_d[:, :])
        nc.gpsimd.dma_start(fcw_last[:], fcw_last_d[:, :])
        nc.gpsimd.dma_start(attn_w[:], attn_w_d[:, :])
        nc.gpsimd.dma_start(c2[:], c2_d[:, :])
        nc.gpsimd.dma_start(demand[:], demand_d[:, :])
        nc.gpsimd.dma_start(dyn[:], cap_d[:, :])
        nc.gpsimd.dma_start(capf[:], capfull_d[:, :])
        attn_k_sb = stage.tile([D, D], F32, tag="wtmp")
        attn_v_sb = stage.tile([D, D], F32, tag="wtmp")
        fc1_sb = stage.tile([D, D], F32, tag="wtmp")
        nc.gpsimd.dma_start(attn_k_sb[:], attn_k_d[:, :])
        nc.gpsimd.dma_start(attn_v_sb[:], attn_v_d[:, :])
        nc.gpsimd.dma_start(fc1_sb[:], fc1_d[:, :])
        nc.gpsimd.dma_start(R_const[:], rconst_d[:, :])
        nc.gpsimd.dma_start(M8[:], m8_d[:, :])
        nc.gpsimd.dma_start(ident[:], ident_d[:, :])
        nc.gpsimd.dma_start(iota_nf[:], iota_d[:, :])
        nc.gpsimd.dma_start(rowbase_i[:], rowbase_d[:, :])
        nc.vector.tensor_scalar_mul(negI[:], ident[:], -1e9)
        nc.vector.memset(onesc[:], 1.0)
        nc.vector.memset(q_bd[:], 0.0)
        nc.vector.memset(lt_nb[:], 0.0)

        # encT via per-n transposes
        for n in range(N):
            st_in = stage.tile([B, D], F32, tag="encstage")
            nc.sync.dma_start(st_in[:], enc_rows[n::N, :])
            ps_t = pshared.tile([B, D], F32, tag="ps_sh")
            nc.tensor.transpose(ps_t[:], st_in[:], ident[:])
            if n % 2 == 0:
                nc.vector.tensor_copy(encT[:, n::N], ps_t[:])
            else:
                nc.scalar.copy(encT[:, n::N], ps_t[:])

        # kT = attn_k^T @ encT
        CH = 512
        for c in range(0, B * N, CH):
            w = min(CH, B * N - c)
            ps_k = pshared.tile([D, CH], F32, tag="ps_sh")
            nc.tensor.matmul(ps_k[:, :w], lhsT=attn_k_sb[:], rhs=encT[:, c:c + w],
                             start=True, stop=True)
            if (c // CH) % 2 == 0:
                nc.vector.tensor_copy(kT[:, c:c + w], ps_k[:, :w])
            else:
                nc.scalar.copy(kT[:, c:c + w], ps_k[:, :w])

        # v_nT per-b
        for b in range(B):
            ps_v = pshared.tile([B, D], F32, tag="ps_sh")
            nc.tensor.matmul(ps_v[:N, :], lhsT=encT[:, b * N:(b + 1) * N],
                             rhs=attn_v_sb[:], start=True, stop=True)
            if b % 2 == 0:
                nc.vector.tensor_copy(v_nT[:N, b * D:(b + 1) * D], ps_v[:N, :])
            else:
                nc.scalar.copy(v_nT[:N, b * D:(b + 1) * D], ps_v[:N, :])

        # pooledT
        pool_sb = stage.tile([B, D], F32, tag="encstage")
        nc.gpsimd.dma_start(pool_sb[:], pool_d[:, :])
        ps_pt = pshared.tile([D, B], F32, tag="ps_sh")
        nc.tensor.transpose(ps_pt[:], pool_sb[:], ident[:])
        poolT_sb = stage.tile([D, B], F32, tag="encstage")
        nc.vector.tensor_copy(poolT_sb[:], ps_pt[:])
        ps_pl = pshared.tile([D, B], F32, tag="ps_sh")
        nc.tensor.matmul(ps_pl[:], lhsT=fc1_sb[:], rhs=poolT_sb[:], start=True,
                         stop=True)
        nc.vector.tensor_copy(pooledT[:], ps_pl[:])

        # state0 = enc[:, 0, :]; dynT0; mask init
        nc.sync.dma_start(state_g[:], enc_rows[0::N, :])
        ps_s0 = pshared.tile([D, B], F32, tag="ps_sh")
        nc.tensor.transpose(ps_s0[:], state_g[:], ident[:])
        nc.vector.tensor_copy(stateT[:], ps_s0[:])
        nc.tensor.transpose(ps_dyT[:], dyn[:], ident[:])
        nc.vector.memset(mask1[:], 0.0)
        nc.vector.tensor_scalar(mask[:], demand[:], dyn[:, 0:1], None, op0=OP.is_gt)
        nc.vector.memset(mask[:, 0:1], 1.0)
        nc.vector.tensor_scalar_mul(neg9mask[:], mask[:], -1e9)
        nc.vector.tensor_copy(ps_dyT_sb[:], ps_dyT[:])

        # ============ STEP ============
        def emit_step(t):
            ps_dec = pshared.tile([D, B], F32, tag="ps_sh")
            nc.tensor.matmul(ps_dec[:], lhsT=fcw_main[:], rhs=stateT[:],
                             start=True, stop=False)
            nc.tensor.matmul(ps_dec[:], lhsT=fcw_last[:], rhs=ps_dyT_sb[:],
                             start=False, stop=True)
            nc.vector.scalar_tensor_tensor(out=dec_inT[:], in0=ps_dec[:], scalar=1.0,
                                           in1=pooledT[:], op0=OP.mult, op1=OP.add)
            ps_q = pshared.tile([D, B], F32, tag="ps_sh")
            nc.tensor.matmul(ps_q[:], lhsT=attn_w[:], rhs=dec_inT[:], start=True,
                             stop=True)
            nc.vector.tensor_tensor(
                q_bd[:].rearrange("p (b h) -> p b h", h=H),
                ps_q[:].to_broadcast([D, B, H]),
                M8[:].rearrange("p (x h) -> p x h", x=1).to_broadcast([D, B, H]),
                op=OP.mult)

            for b in range(B):
                nc.tensor.matmul(psc[:N, b * H:(b + 1) * H],
                                 lhsT=kT[:, b * N:(b + 1) * N],
                                 rhs=q_bd[:, b * H:(b + 1) * H],
                                 start=(b % 64 == 0), stop=False,
                                 skip_group_check=True)
            for c in range(2):
                rhs_ap = negI[:, c * 64:(c + 1) * 64].to_broadcast([B, 64, H])
                nc.tensor.matmul(psc[:N, c * 512:(c + 1) * 512],
                                 lhsT=mask[:], rhs=rhs_ap,
                                 start=False, stop=True, skip_group_check=True)
            nc.scalar.activation(e_sb[:N, :], psc[:N, :], AF.Exp, scale=INV_HD)
            e_h = e_sb[:N, :].rearrange("p (b h) -> p h b", h=H)
            psS = pshared.tile([B, H], F32, tag="ps_sh")
            for h in range(H):
                nc.tensor.matmul(psS[:, h:h + 1], lhsT=e_h[:, h, :], rhs=onesc[:N, :],
                                 start=(h == 0), stop=(h == H - 1),
                                 skip_group_check=True)
            nc.vector.reciprocal(Sinv_sb[:], psS[:])
            ps_sT = pshared.tile([H, B], F32, tag="ps_sh")
            nc.tensor.transpose(ps_sT[:], Sinv_sb[:], ident[:])
            nc.vector.tensor_copy(SinvT_sb[:], ps_sT[:])
            nc.tensor.matmul(ps_rep[:], lhsT=R_const[:], rhs=SinvT_sb[:],
                             start=True, stop=True)

            for b in range(B):
                nc.tensor.matmul(pso[:, b * H:(b + 1) * H],
                                 lhsT=v_nT[:N, b * D:(b + 1) * D],
                                 rhs=e_sb[:N, b * H:(b + 1) * H],
                                 start=(b % 64 == 0), stop=(b % 64 == 63),
                                 skip_group_check=True)
            nc.vector.tensor_tensor(
                q_bd[:].rearrange("p (b h) -> p b h", h=H),
                pso[:].rearrange("p (b h) -> p b h", h=H),
                M8[:].rearrange("p (x h) -> p x h", x=1).to_broadcast([D, B, H]),
                op=OP.mult)
            nc.vector.reduce_sum(outT_u[:],
                                 q_bd[:].rearrange("p (b h) -> p b h", h=H),
                                 axis=mybir.AxisListType.X)
            nc.vector.tensor_tensor(outT_s[:], outT_u[:], ps_rep[:], op=OP.mult)

            ps_u = pshared.tile([D, B], F32, tag="ps_sh")
            nc.tensor.matmul(ps_u[:], lhsT=c2[:], rhs=outT_s[:], start=True, stop=True)
            nc.vector.tensor_copy(u_sb[:], ps_u[:])
            ps_lg = pshared.tile([B, B], F32, tag="ps_sh")
            for b in range(B):
                nc.tensor.matmul(ps_lg[:N, b:b + 1],
                                 lhsT=encT[:, b * N:(b + 1) * N],
                                 rhs=u_sb[:, b:b + 1],
                                 start=(b == 0), stop=(b == B - 1),
                                 skip_group_check=True)
            nc.vector.tensor_copy(lt_nb[:N, :], ps_lg[:N, :])
            ps_lt = pshared.tile([B, B], F32, tag="ps_sh")
            nc.tensor.transpose(ps_lt[:], lt_nb[:], ident[:])
            nc.scalar.activation(ltanh[:], ps_lt[:, :N], AF.Tanh, scale=INV_D)
            nc.vector.scalar_tensor_tensor(out=logits_sb[:], in0=ltanh[:], scalar=10.0,
                                           in1=neg9mask[:], op0=OP.mult, op1=OP.add)

            nc.vector.max(out=max8[:], in_=logits_sb[:])
            nc.vector.max_index(out=idx8u[:], in_max=max8[:], in_values=logits_sb[:])
            nc.vector.tensor_copy(idx_f[:], idx8u[:, 0:1])
            nc.vector.tensor_copy(idx_i[:], idx8u[:, 0:1])
            nc.vector.tensor_scalar(onehot[:], iota_nf[:], idx_f[:, 0:1], None,
                                    op0=OP.is_equal)
            nc.vector.tensor_scalar_mul(neg_lmax[:], max8[:, 0:1], -1.0)
            nc.scalar.activation(scr101[:], logits_sb[:], AF.Exp,
                                 bias=neg_lmax[:, 0:1], accum_out=sume[:])
            nc.vector.reduce_sum(sm1_old[:], mask1[:, 1:N], axis=mybir.AxisListType.X)
            nc.vector.tensor_scalar(nd_old[:], sm1_old[:], float(N - 1), None,
                                    op0=OP.is_lt)
            nc.vector.tensor_scalar(nd_old_i[:], sm1_old[:], float(N - 1), None,
                                    op0=OP.is_lt)
            nc.vector.select(Wsel[:], nd_old_i[:], sume[:], onesc[:])
            nc.vector.tensor_copy(W_buf[:, bass.ds(t, 1)], Wsel[:])
            nc.vector.tensor_copy(act_buf[:, bass.ds(t, 1)], idx_i[:])
            nc.vector.tensor_tensor(scr101[:], onehot[:], demand[:], op=OP.mult)
            nc.vector.reduce_sum(sd[:], scr101[:], axis=mybir.AxisListType.X)
            nc.vector.tensor_scalar(gd[:], idx_f[:], 0.5, None, op0=OP.is_lt)
            nc.vector.tensor_scalar(gd_i[:], idx_f[:], 0.5, None, op0=OP.is_lt)
            nc.vector.tensor_tensor(dmg[:], dyn[:], sd[:], op=OP.subtract)
            nc.vector.select(dyn[:], gd_i[:], capf[:], dmg[:])
            nc.vector.tensor_tensor(mx[:], mask1[:], onehot[:], op=OP.max)
            nc.vector.select(scr101[:], gd_i[:, 0:1].to_broadcast([B, N]), mask1[:],
                             mx[:])
            nc.vector.tensor_copy(mask1[:], scr101[:])
            nc.vector.reduce_sum(sm1_new[:], mask1[:, 1:N], axis=mybir.AxisListType.X)
            nc.vector.tensor_scalar(nd_new[:], sm1_new[:], float(N - 1) - 0.5, None,
                                    op0=OP.is_le)
            nc.vector.tensor_scalar(dgt[:], demand[:], dyn[:, 0:1], None, op0=OP.is_gt)
            nc.vector.tensor_tensor(mask[:], mask1[:], dgt[:], op=OP.max)
            nc.vector.tensor_tensor(dep[:], gd[:], nd_new[:], op=OP.mult)
            nc.vector.tensor_copy(mask[:, 0:1], dep[:])
            nc.vector.tensor_scalar_mul(neg9mask[:], mask[:], -1e9)

            nc.vector.tensor_tensor(gidx_i[:], idx_i[:], rowbase_i[:], op=OP.add)
            nc.gpsimd.indirect_dma_start(
                out=state_g[:], out_offset=None, in_=enc_rows[:, :],
                in_offset=bass.IndirectOffsetOnAxis(ap=gidx_i[:, 0:1], axis=0))
            ps_st = pshared.tile([D, B], F32, tag="ps_sh")
            nc.tensor.transpose(ps_st[:], state_g[:], ident[:])
            nc.vector.tensor_copy(stateT[:], ps_st[:])
            nc.tensor.transpose(ps_dyT[:], dyn[:], ident[:])
            nc.vector.tensor_copy(ps_dyT_sb[:], ps_dyT[:])

        if unrolled:
            for t in range(n_steps):
                emit_step(t)
        else:
            with tc.For_i(0, n_steps, 1,
                          hint_engines=(mybir.EngineType.PE,)) as t:
                emit_step(t)

        # ============ TAIL ============
        nc.scalar.activation(lnW[:], W_buf[:], AF.Ln)
        nc.vector.reduce_sum(sumln[:], lnW[:], axis=mybir.AxisListType.X)
        nc.vector.tensor_scalar_mul(logp_out[:], sumln[:], -1.0)
        nc.gpsimd.dma_start(logps_d[:, :], logp_out[:])
        nc.gpsimd.dma_start(actions_d[:, :], act_buf[:])

    nc.compile()
    return nc


def _get_program(n_steps: int):
    with _cache_lock:
        if n_steps not in _cache:
            _cache[n_steps] = _build(n_steps)
        return _cache[n_steps]


def _make_in_maps(inputs):
    enc = np.ascontiguousarray(np.asarray(inputs["encoder_inputs"], np.float32))
    pool = np.asarray(inputs["pool"], np.float32)
    cap = np.asarray(inputs["capcity"], np.float32)
    dem = np.asarray(inputs["demand"], np.float32)
    fc_w = np.asarray(inputs["fc_w"], np.float32)
    fc1_w = np.ascontiguousarray(np.asarray(inputs["fc1_w"], np.float32))
    attn_w = np.ascontiguousarray(np.asarray(inputs["attn_w"], np.float32))
    attn_k = np.ascontiguousarray(np.asarray(inputs["attn_k"], np.float32))
    attn_v = np.ascontiguousarray(np.asarray(inputs["attn_v"], np.float32))
    attn_fc = np.asarray(inputs["attn_fc"], np.float32)
    prob_k = np.asarray(inputs["prob_k"], np.float32)
    c2 = np.ascontiguousarray(attn_fc @ prob_k.T)
    cap_full = np.full((B, 1), cap[0, 0], np.float32)
    shared = {
        "fc_w_main": np.ascontiguousarray(fc_w[:D]),
        "fc_w_last": np.ascontiguousarray(fc_w[D:D + 1]),
        "fc1_w": fc1_w, "attn_w": attn_w, "attn_k": attn_k,
        "attn_v": attn_v, "c2": c2, "capfull": cap_full,
        "r_const": np.repeat(np.eye(H, dtype=np.float32), HD, axis=1),
        "m8": np.repeat(np.eye(H, dtype=np.float32), HD, axis=0),
        "ident_in": np.eye(B, dtype=np.float32),
        "iota_in": np.tile(np.arange(N, dtype=np.float32), (B, 1)),
        "rowbase_in": (np.arange(B, dtype=np.int32) * N)[:, None],
    }
    maps = []
    for c in range(N_CORES):
        s = slice(c * B, (c + 1) * B)
        m = dict(shared)
        m["enc_rows"] = np.ascontiguousarray(enc[s].reshape(B * N, D))
        m["demand"] = np.ascontiguousarray(dem[s])
        m["capcity"] = np.ascontiguousarray(cap[s])
        m["pool"] = np.ascontiguousarray(pool[s])
        maps.append(m)
    return maps


def kernel(**inputs):
    n_steps = int(np.asarray(inputs.get("n_steps", 142)))
    assert int(np.asarray(inputs.get("num_depots", 1))) == 1
    assert int(np.asarray(inputs.get("temperature", 1))) == 1
    assert int(np.asarray(inputs.get("greedy", 1))) == 1
    Btot = np.asarray(inputs["encoder_inputs"]).shape[0]
    assert Btot == B * N_CORES

    from concourse.bass_utils import run_bass_kernel_spmd

    nc = _get_program(n_steps)
    maps = _make_in_maps(inputs)
    res = run_bass_kernel_spmd(nc, maps, core_ids=list(range(N_CORES)))
    actions = np.concatenate(
        [res.results[c]["actions"] for c in range(N_CORES)], axis=0).astype(np.int32)
    logps = np.concatenate(
        [res.results[c]["logps"][:, 0] for c in range(N_CORES)], axis=0
    ).astype(np.float32)
    return actions, logps


# revision 3
# speedup vs baseline: 1.0080x; 1.0080x over previous
"""Trainium2 Bass kernel for nn_Decoder1 (CVRP pointer-network greedy decoder).

Self-contained: builds one SPMD Bass program (8 cores, batch-parallel:
1024 rows -> 128 per core), runs it via run_bass_kernel_spmd, and
reassembles full outputs (actions int32 [1024,142], logps f32 [1024]).

Per-core design (B=128 batch rows live on the 128 SBUF partitions for all
decode bookkeeping; the three per-row attention contractions run as per-row
PE matmuls against SBUF-resident transposed encodings):

  encT [d, (b,n)]  = enc^T                (logits stationary + k/v source)
  kT   [dout,(b,n)] = (enc@attn_k)^T      (compat stationary)
  v_nT [n, (b,dout)] = enc@attn_v         (out-contraction stationary)

Step: dec_inT/qT dense matmuls -> per-b compat matmuls into PSUM [n,(b,h)]
with the -1e9 mask folded in via one mask @ (-1e9*I kron 1_8) accumulate
matmul -> exp on ACT (no max-subtract needed; masked lanes underflow to 0)
-> 8 ones-matmuls give softmax denominators -> per-b out-contraction
(cross-head [dout,(b,h')] in PSUM, diagonal extracted + 1/S scaled with
mask-multiply + reduce) -> u = (attn_fc@prob_k^T)^T @ outT -> per-b logits
matmuls [n,b] -> PE transpose -> tanh/mask -> argmax via DVE max/max_index
-> mask/dyn/logp bookkeeping -> next state via indirect-DMA row gather.
All ln() calls deferred to one batched ACT call after the loop (keeps the
ACT table set fixed at exp/tanh in-loop); logp = -sum_t ln(W_t).
"""

import os
import sys
import threading
from contextlib import ExitStack

import numpy as np

for _p in ("/opt/trn_rl_repo", "/root/.axon_site/_ro/trn_rl_repo"):
    if os.path.isdir(_p) and _p not in sys.path:
        sys.path.insert(0, _p)

import concourse.bass as bass
import concourse.mybir as mybir
from concourse import bacc
from concourse.tile import TileContext

F32 = mybir.dt.float32
I32 = mybir.dt.int32
U32 = mybir.dt.uint32
OP = mybir.AluOpType
AF = mybir.ActivationFunctionType

B = 128      # batch rows per core
N = 101      # nodes
D = 128      # model dim
H = 8        # heads
HD = D // H
INV_HD = float(1.0 / np.sqrt(HD))
INV_D = float(1.0 / np.sqrt(D))
N_CORES = 8

_cache = {}
_cache_lock = threading.Lock()


def _build(n_steps: int, unrolled: bool = False):
    nc = bacc.Bacc("TRN2", target_bir_lowering=False, debug=False,
                   num_devices=N_CORES)

    enc_rows = nc.dram_tensor("enc_rows", [B * N, D], F32, kind="ExternalInput")
    demand_d = nc.dram_tensor("demand", [B, N], F32, kind="ExternalInput")
    cap_d = nc.dram_tensor("capcity", [B, 1], F32, kind="ExternalInput")
    capfull_d = nc.dram_tensor("capfull", [B, 1], F32, kind="ExternalInput")
    pool_d = nc.dram_tensor("pool", [B, D], F32, kind="ExternalInput")
    fcw_main_d = nc.dram_tensor("fc_w_main", [D, D], F32, kind="ExternalInput")
    fcw_last_d = nc.dram_tensor("fc_w_last", [1, D], F32, kind="ExternalInput")
    fc1_d = nc.dram_tensor("fc1_w", [D, D], F32, kind="ExternalInput")
    attn_w_d = nc.dram_tensor("attn_w", [D, D], F32, kind="ExternalInput")
    attn_k_d = nc.dram_tensor("attn_k", [D, D], F32, kind="ExternalInput")
    attn_v_d = nc.dram_tensor("attn_v", [D, D], F32, kind="ExternalInput")
    c2_d = nc.dram_tensor("c2", [D, D], F32, kind="ExternalInput")
    rconst_d = nc.dram_tensor("r_const", [H, B], F32, kind="ExternalInput")
    m8_d = nc.dram_tensor("m8", [B, H], F32, kind="ExternalInput")
    ident_d = nc.dram_tensor("ident_in", [B, B], F32, kind="ExternalInput")
    iota_d = nc.dram_tensor("iota_in", [B, N], F32, kind="ExternalInput")
    rowbase_d = nc.dram_tensor("rowbase_in", [B, 1], I32, kind="ExternalInput")

    actions_d = nc.dram_tensor("actions", [B, n_steps], I32, kind="ExternalOutput")
    logps_d = nc.dram_tensor("logps", [B, 1], F32, kind="ExternalOutput")

    with TileContext(nc) as tc, ExitStack() as ctx:
        res = ctx.enter_context(tc.tile_pool(name="res", bufs=1))
        stage = ctx.enter_context(tc.tile_pool(name="stage", bufs=3))
        pp = ctx.enter_context(tc.tile_pool(name="pp", bufs=1, space="PSUM"))
        pshared = ctx.enter_context(tc.tile_pool(name="pshared", bufs=2, space="PSUM"))

        encT = res.tile([D, B * N], F32)
        kT = res.tile([D, B * N], F32)
        v_nT = res.tile([B, B * D], F32)
        e_sb = res.tile([B, B * H], F32)
        q_bd = res.tile([D, B * H], F32)
        negI = res.tile([B, B], F32)
        ident = res.tile([B, B], F32)
        fcw_main = res.tile([D, D], F32)
        fcw_last = res.tile([1, D], F32)
        attn_w = res.tile([D, D], F32)
        c2 = res.tile([D, D], F32)
        pooledT = res.tile([D, B], F32)
        demand = res.tile([B, N], F32)
        mask = res.tile([B, N], F32)
        mask1 = res.tile([B, N], F32)
        neg9mask = res.tile([B, N], F32)
        onehot = res.tile([B, N], F32)
        dgt = res.tile([B, N], F32)
        mx = res.tile([B, N], F32)
        scr101 = res.tile([B, N], F32)
        iota_nf = res.tile([B, N], F32)
        rowbase_i = res.tile([B, 1], I32)
        dyn = res.tile([B, 1], F32)
        capf = res.tile([B, 1], F32)
        onesc = res.tile([B, 1], F32)
        R_const = res.tile([H, B], F32)
        M8 = res.tile([B, H], F32)
        stateT = res.tile([D, B], F32)
        state_g = res.tile([B, D], F32)
        dec_inT = res.tile([D, B], F32)
        outT_u = res.tile([D, B], F32)
        outT_s = res.tile([D, B], F32)
        u_sb = res.tile([D, B], F32)
        lt_nb = res.tile([B, B], F32)
        ltanh = res.tile([B, N], F32)
        logits_sb = res.tile([B, N], F32)
        Sinv_sb = res.tile([B, H], F32)
        SinvT_sb = res.tile([H, B], F32)
        max8 = res.tile([B, 8], F32)
        idx8u = res.tile([B, 8], U32)
        idx_f = res.tile([B, 1], F32)
        idx_i = res.tile([B, 1], I32)
        gidx_i = res.tile([B, 1], I32)
        neg_lmax = res.tile([B, 1], F32)
        sume = res.tile([B, 1], F32)
        sm1_old = res.tile([B, 1], F32)
        nd_old = res.tile([B, 1], F32)
        sm1_new = res.tile([B, 1], F32)
        nd_new = res.tile([B, 1], F32)
        sd = res.tile([B, 1], F32)
        gd = res.tile([B, 1], F32)
        gd_i = res.tile([B, 1], I32)
        nd_old_i = res.tile([B, 1], I32)
        dmg = res.tile([B, 1], F32)
        dep = res.tile([B, 1], F32)
        Wsel = res.tile([B, 1], F32)
        W_buf = res.tile([B, n_steps], F32)
        lnW = res.tile([B, n_steps], F32)
        sumln = res.tile([B, 1], F32)
        logp_out = res.tile([B, 1], F32)
        act_buf = res.tile([B, n_steps], I32)
        ps_dyT_sb = res.tile([1, B], F32)

        psc = pp.tile([B, B * H], F32)
        pso = pp.tile([B, B * H], F32)
        ps_rep = pp.tile([D, B], F32)
        ps_dyT = pp.tile([1, B], F32)

        # ============ INIT ============
        nc.gpsimd.dma_start(fcw_main[:], fcw_main_d[:, :])
        nc.gpsimd.dma_start(fcw_last[:], fcw_last_d[:, :])
        nc.gpsimd.dma_start(attn_w[:], attn_w_d[:, :])
        nc.gpsimd.dma_start(c2[:], c2_d[:, :])
        nc.gpsimd.dma_start(demand[:], demand_d[:, :])
        nc.gpsimd.dma_start(dyn[:], cap_d[:, :])
        nc.gpsimd.dma_start(capf[:], capfull_d[:, :])
        attn_k_sb = stage.tile([D, D], F32, tag="wtmp")
        attn_v_sb = stage.tile([D, D], F32, tag="wtmp")
        fc1_sb = stage.tile([D, D], F32, tag="wtmp")
        nc.gpsimd.dma_start(attn_k_sb[:], attn_k_d[:, :])
        nc.gpsimd.dma_start(attn_v_sb[:], attn_v_d[:, :])
        nc.gpsimd.dma_start(fc1_sb[:], fc1_d[:, :])
        nc.gpsimd.dma_start(R_const[:], rconst_d[:, :])
        nc.gpsimd.dma_start(M8[:], m8_d[:, :])
        nc.gpsimd.dma_start(ident[:], ident_d[:, :])
        nc.gpsimd.dma_start(iota_nf[:], iota_d[:, :])
        nc.gpsimd.dma_start(rowbase_i[:], rowbase_d[:, :])
        nc.vector.tensor_scalar_mul(negI[:], ident[:], -1e9)
        nc.vector.memset(onesc[:], 1.0)
        nc.vector.memset(q_bd[:], 0.0)
        nc.vector.memset(lt_nb[:], 0.0)

        # encT via per-n transposes
        for n in range(N):
            st_in = stage.tile([B, D], F32, tag="encstage")
            nc.sync.dma_start(st_in[:], enc_rows[n::N, :])
            ps_t = pshared.tile([B, D], F32, tag="ps_sh")
            nc.tensor.transpose(ps_t[:], st_in[:], ident[:])
            if n % 2 == 0:
                nc.vector.tensor_copy(encT[:, n::N], ps_t[:])
            else:
                nc.scalar.copy(encT[:, n::N], ps_t[:])

        # kT = attn_k^T @ encT
        CH = 512
        for c in range(0, B * N, CH):
            w = min(CH, B * N - c)
            ps_k = pshared.tile([D, CH], F32, tag="ps_sh")
            nc.tensor.matmul(ps_k[:, :w], lhsT=attn_k_sb[:], rhs=encT[:, c:c + w],
                             start=True, stop=True)
            if (c // CH) % 2 == 0:
                nc.vector.tensor_copy(kT[:, c:c + w], ps_k[:, :w])
            else:
                nc.scalar.copy(kT[:, c:c + w], ps_k[:, :w])

        # v_nT per-b
        for b in range(B):
            ps_v = pshared.tile([B, D], F32, tag="ps_sh")
            nc.tensor.matmul(ps_v[:N, :], lhsT=encT[:, b * N:(b + 1) * N],
                             rhs=attn_v_sb[:], start=True, stop=True)
            if b % 2 == 0:
                nc.vector.tensor_copy(v_nT[:N, b * D:(b + 1) * D], ps_v[:N, :])
            else:
                nc.scalar.copy(v_nT[:N, b * D:(b + 1) * D], ps_v[:N, :])

        # pooledT
        pool_sb = stage.tile([B, D], F32, tag="encstage")
        nc.gpsimd.dma_start(pool_sb[:], pool_d[:, :])
        ps_pt = pshared.tile([D, B], F32, tag="ps_sh")
        nc.tensor.transpose(ps_pt[:], pool_sb[:], ident[:])
        poolT_sb = stage.tile([D, B], F32, tag="encstage")
        nc.vector.tensor_copy(poolT_sb[:], ps_pt[:])
        ps_pl = pshared.tile([D, B], F32, tag="ps_sh")
        nc.tensor.matmul(ps_pl[:], lhsT=fc1_sb[:], rhs=poolT_sb[:], start=True,
                         stop=True)
        nc.vector.tensor_copy(pooledT[:], ps_pl[:])

        # state0 = enc[:, 0, :]; dynT0; mask init
        nc.sync.dma_start(state_g[:], enc_rows[0::N, :])
        ps_s0 = pshared.tile([D, B], F32, tag="ps_sh")
        nc.tensor.transpose(ps_s0[:], state_g[:], ident[:])
        nc.vector.tensor_copy(stateT[:], ps_s0[:])
        nc.tensor.transpose(ps_dyT[:], dyn[:], ident[:])
        nc.vector.memset(mask1[:], 0.0)
        nc.vector.tensor_scalar(mask[:], demand[:], dyn[:, 0:1], None, op0=OP.is_gt)
        nc.vector.memset(mask[:, 0:1], 1.0)
        nc.vector.tensor_scalar_mul(neg9mask[:], mask[:], -1e9)
        nc.vector.tensor_copy(ps_dyT_sb[:], ps_dyT[:])

        # ============ STEP ============
        def emit_step(t):
            ps_dec = pshared.tile([D, B], F32, tag="ps_sh")
            nc.tensor.matmul(ps_dec[:], lhsT=fcw_main[:], rhs=stateT[:],
                             start=True, stop=False)
            nc.tensor.matmul(ps_dec[:], lhsT=fcw_last[:], rhs=ps_dyT_sb[:],
                             start=False, stop=True)
            nc.vector.scalar_tensor_tensor(out=dec_inT[:], in0=ps_dec[:], scalar=1.0,
                                           in1=pooledT[:], op0=OP.mult, op1=OP.add)
            ps_q = pshared.tile([D, B], F32, tag="ps_sh")
            nc.tensor.matmul(ps_q[:], lhsT=attn_w[:], rhs=dec_inT[:], start=True,
                             stop=True)
            nc.vector.tensor_tensor(
                q_bd[:].rearrange("p (b h) -> p b h", h=H),
                ps_q[:].to_broadcast([D, B, H]),
                M8[:].rearrange("p (x h) -> p x h", x=1).to_broadcast([D, B, H]),
                op=OP.mult)

            for b in range(B):
                nc.tensor.matmul(psc[:N, b * H:(b + 1) * H],
                                 lhsT=kT[:, b * N:(b + 1) * N],
                                 rhs=q_bd[:, b * H:(b + 1) * H],
                                 start=(b % 64 == 0), stop=False,
                                 skip_group_check=True)
            for c in range(2):
                rhs_ap = negI[:, c * 64:(c + 1) * 64].to_broadcast([B, 64, H])
                nc.tensor.matmul(psc[:N, c * 512:(c + 1) * 512],
                                 lhsT=mask[:], rhs=rhs_ap,
                                 start=False, stop=True, skip_group_check=True)
            nc.scalar.activation(e_sb[:N, :], psc[:N, :], AF.Exp, scale=INV_HD)
            for b in range(B):
                nc.tensor.matmul(pso[:, b * H:(b + 1) * H],
                                 lhsT=v_nT[:N, b * D:(b + 1) * D],
                                 rhs=e_sb[:N, b * H:(b + 1) * H],
                                 start=(b % 64 == 0), stop=(b % 64 == 63),
                                 skip_group_check=True)
            e_h = e_sb[:N, :].rearrange("p (b h) -> p h b", h=H)
            psS = pshared.tile([B, H], F32, tag="ps_sh")
            for h in range(H):
                nc.tensor.matmul(psS[:, h:h + 1], lhsT=e_h[:, h, :], rhs=onesc[:N, :],
                                 start=(h == 0), stop=(h == H - 1),
                                 skip_group_check=True)
            nc.vector.reciprocal(Sinv_sb[:], psS[:])
            ps_sT = pshared.tile([H, B], F32, tag="ps_sh")
            nc.tensor.transpose(ps_sT[:], Sinv_sb[:], ident[:])
            nc.vector.tensor_copy(SinvT_sb[:], ps_sT[:])
            nc.tensor.matmul(ps_rep[:], lhsT=R_const[:], rhs=SinvT_sb[:],
                             start=True, stop=True)

            nc.vector.tensor_tensor(
                q_bd[:].rearrange("p (b h) -> p b h", h=H),
                pso[:].rearrange("p (b h) -> p b h", h=H),
                M8[:].rearrange("p (x h) -> p x h", x=1).to_broadcast([D, B, H]),
                op=OP.mult)
            nc.vector.reduce_sum(outT_u[:],
                                 q_bd[:].rearrange("p (b h) -> p b h", h=H),
                                 axis=mybir.AxisListType.X)
            nc.vector.tensor_tensor(outT_s[:], outT_u[:], ps_rep[:], op=OP.mult)

            ps_u = pshared.tile([D, B], F32, tag="ps_sh")
            nc.tensor.matmul(ps_u[:], lhsT=c2[:], rhs=outT_s[:], start=True, stop=True)
            nc.vector.tensor_copy(u_sb[:], ps_u[:])
            ps_lg = pshared.tile([B, B], F32, tag="ps_sh")
            for b in range(B):
                nc.tensor.matmul(ps_lg[:N, b:b + 1],
                                 lhsT=encT[:, b * N:(b + 1) * N],
                                 rhs=u_sb[:, b:b + 1],
                                 start=(b == 0), stop=(b == B - 1),
                                 skip_group_check=True)
            nc.vector.tensor_copy(lt_nb[:N, :], ps_lg[:N, :])
            ps_lt = pshared.tile([B, B], F32, tag="ps_sh")
            nc.tensor.transpose(ps_lt[:], lt_nb[:], ident[:])
            nc.scalar.activation(ltanh[:], ps_lt[:, :N], AF.Tanh, scale=INV_D)
            nc.vector.scalar_tensor_tensor(out=logits_sb[:], in0=ltanh[:], scalar=10.0,
                                           in1=neg9mask[:], op0=OP.mult, op1=OP.add)

            nc.vector.max(out=max8[:], in_=logits_sb[:])
            nc.vector.max_index(out=idx8u[:], in_max=max8[:], in_values=logits_sb[:])
            nc.vector.tensor_copy(idx_f[:], idx8u[:, 0:1])
            nc.vector.tensor_copy(idx_i[:], idx8u[:, 0:1])
            nc.vector.tensor_tensor(gidx_i[:], idx_i[:], rowbase_i[:], op=OP.add)
            nc.gpsimd.indirect_dma_start(
                out=state_g[:], out_offset=None, in_=enc_rows[:, :],
                in_offset=bass.IndirectOffsetOnAxis(ap=gidx_i[:, 0:1], axis=0))
            ps_st = pshared.tile([D, B], F32, tag="ps_sh")
            nc.tensor.transpose(ps_st[:], state_g[:], ident[:])
            nc.vector.tensor_copy(stateT[:], ps_st[:])
            nc.vector.tensor_scalar(onehot[:], iota_nf[:], idx_f[:, 0:1], None,
                                    op0=OP.is_equal)
            nc.vector.tensor_scalar_mul(neg_lmax[:], max8[:, 0:1], -1.0)
            nc.scalar.activation(scr101[:], logits_sb[:], AF.Exp,
                                 bias=neg_lmax[:, 0:1], accum_out=sume[:])
            nc.vector.reduce_sum(sm1_old[:], mask1[:, 1:N], axis=mybir.AxisListType.X)
            nc.vector.tensor_scalar(nd_old[:], sm1_old[:], float(N - 1), None,
                                    op0=OP.is_lt)
            nc.vector.tensor_scalar(nd_old_i[:], sm1_old[:], float(N - 1), None,
                                    op0=OP.is_lt)
            nc.vector.select(Wsel[:], nd_old_i[:], sume[:], onesc[:])
            nc.vector.tensor_copy(W_buf[:, bass.ds(t, 1)], Wsel[:])
            nc.vector.tensor_copy(act_buf[:, bass.ds(t, 1)], idx_i[:])
            nc.vector.tensor_tensor(scr101[:], onehot[:], demand[:], op=OP.mult)
            nc.vector.reduce_sum(sd[:], scr101[:], axis=mybir.AxisListType.X)
            nc.vector.tensor_scalar(gd[:], idx_f[:], 0.5, None, op0=OP.is_lt)
            nc.vector.tensor_scalar(gd_i[:], idx_f[:], 0.5, None, op0=OP.is_lt)
            nc.vector.tensor_tensor(dmg[:], dyn[:], sd[:], op=OP.subtract)
            nc.vector.select(dyn[:], gd_i[:], capf[:], dmg[:])
            nc.vector.tensor_tensor(mx[:], mask1[:], onehot[:], op=OP.max)
            nc.vector.select(scr101[:], gd_i[:, 0:1].to_broadcast([B, N]), mask1[:],
                             mx[:])
            nc.vector.tensor_copy(mask1[:], scr101[:])
            nc.vector.reduce_sum(sm1_new[:], mask1[:, 1:N], axis=mybir.AxisListType.X)
            nc.vector.tensor_scalar(nd_new[:], sm1_new[:], float(N - 1) - 0.5, None,
                                    op0=OP.is_le)
            nc.vector.tensor_scalar(dgt[:], demand[:], dyn[:, 0:1], None, op0=OP.is_gt)
            nc.vector.tensor_tensor(mask[:], mask1[:], dgt[:], op=OP.max)
            nc.vector.tensor_tensor(dep[:], gd[:], nd_new[:], op=OP.mult)
            nc.vector.tensor_copy(mask[:, 0:1], dep[:])
            nc.vector.tensor_scalar_mul(neg9mask[:], mask[:], -1e9)

            nc.tensor.transpose(ps_dyT[:], dyn[:], ident[:])
            nc.vector.tensor_copy(ps_dyT_sb[:], ps_dyT[:])

        if unrolled:
            for t in range(n_steps):
                emit_step(t)
        else:
            with tc.For_i(0, n_steps, 1,
                          hint_engines=(mybir.EngineType.PE,)) as t:
                emit_step(t)

        # ============ TAIL ============
        nc.scalar.activation(lnW[:], W_buf[:], AF.Ln)
        nc.vector.reduce_sum(sumln[:], lnW[:], axis=mybir.AxisListType.X)
        nc.vector.tensor_scalar_mul(logp_out[:], sumln[:], -1.0)
        nc.gpsimd.dma_start(logps_d[:, :], logp_out[:])
        nc.gpsimd.dma_start(actions_d[:, :], act_buf[:])

    nc.compile()
    return nc


def _get_program(n_steps: int):
    with _cache_lock:
        if n_steps not in _cache:
            _cache[n_steps] = _build(n_steps)
        return _cache[n_steps]


def _make_in_maps(inputs):
    enc = np.ascontiguousarray(np.asarray(inputs["encoder_inputs"], np.float32))
    pool = np.asarray(inputs["pool"], np.float32)
    cap = np.asarray(inputs["capcity"], np.float32)
    dem = np.asarray(inputs["demand"], np.float32)
    fc_w = np.asarray(inputs["fc_w"], np.float32)
    fc1_w = np.ascontiguousarray(np.asarray(inputs["fc1_w"], np.float32))
    attn_w = np.ascontiguousarray(np.asarray(inputs["attn_w"], np.float32))
    attn_k = np.ascontiguousarray(np.asarray(inputs["attn_k"], np.float32))
    attn_v = np.ascontiguousarray(np.asarray(inputs["attn_v"], np.float32))
    attn_fc = np.asarray(inputs["attn_fc"], np.float32)
    prob_k = np.asarray(inputs["prob_k"], np.float32)
    c2 = np.ascontiguousarray(attn_fc @ prob_k.T)
    cap_full = np.full((B, 1), cap[0, 0], np.float32)
    shared = {
        "fc_w_main": np.ascontiguousarray(fc_w[:D]),
        "fc_w_last": np.ascontiguousarray(fc_w[D:D + 1]),
        "fc1_w": fc1_w, "attn_w": attn_w, "attn_k": attn_k,
        "attn_v": attn_v, "c2": c2, "capfull": cap_full,
        "r_const": np.repeat(np.eye(H, dtype=np.float32), HD, axis=1),
        "m8": np.repeat(np.eye(H, dtype=np.float32), HD, axis=0),
        "ident_in": np.eye(B, dtype=np.float32),
        "iota_in": np.tile(np.arange(N, dtype=np.float32), (B, 1)),
        "rowbase_in": (np.arange(B, dtype=np.int32) * N)[:, None],
    }
    maps = []
    for c in range(N_CORES):
        s = slice(c * B, (c + 1) * B)
        m = dict(shared)
        m["enc_rows"] = np.ascontiguousarray(enc[s].reshape(B * N, D))
        m["demand"] = np.ascontiguousarray(dem[s])
        m["capcity"] = np.ascontiguousarray(cap[s])
        m["pool"] = np.ascontiguousarray(pool[s])
        maps.append(m)
    return maps


def kernel(**inputs):
    n_steps = int(np.asarray(inputs.get("n_steps", 142)))
    assert int(np.asarray(inputs.get("num_depots", 1))) == 1
    assert int(np.asarray(inputs.get("temperature", 1))) == 1
    assert int(np.asarray(inputs.get("greedy", 1))) == 1
    Btot = np.asarray(inputs["encoder_inputs"]).shape[0]
    assert Btot == B * N_CORES

    from concourse.bass_utils import run_bass_kernel_spmd

    nc = _get_program(n_steps)
    maps = _make_in_maps(inputs)
    res = run_bass_kernel_spmd(nc, maps, core_ids=list(range(N_CORES)))
    actions = np.concatenate(
        [res.results[c]["actions"] for c in range(N_CORES)], axis=0).astype(np.int32)
    logps = np.concatenate(
        [res.results[c]["logps"][:, 0] for c in range(N_CORES)], axis=0
    ).astype(np.float32)
    return actions, logps


# revision 4
# speedup vs baseline: 1.0495x; 1.0412x over previous
"""Trainium2 Bass kernel for nn_Decoder1 (CVRP pointer-network greedy decoder).

Self-contained: builds one SPMD Bass program (8 cores, batch-parallel:
1024 rows -> 128 per core), runs it via run_bass_kernel_spmd, and
reassembles full outputs (actions int32 [1024,142], logps f32 [1024]).

Per-core design (B=128 batch rows live on the 128 SBUF partitions for all
decode bookkeeping; the three per-row attention contractions run as per-row
PE matmuls against SBUF-resident transposed encodings):

  encT [d, (b,n)]  = enc^T                (logits stationary + k/v source)
  kT   [dout,(b,n)] = (enc@attn_k)^T      (compat stationary)
  v_nT [n, (b,dout)] = enc@attn_v         (out-contraction stationary)

Step: dec_inT/qT dense matmuls -> per-b compat matmuls into PSUM [n,(b,h)]
with the -1e9 mask folded in via one mask @ (-1e9*I kron 1_8) accumulate
matmul -> exp on ACT (no max-subtract needed; masked lanes underflow to 0)
-> 8 ones-matmuls give softmax denominators -> per-b out-contraction
(cross-head [dout,(b,h')] in PSUM, diagonal extracted + 1/S scaled with
mask-multiply + reduce) -> u = (attn_fc@prob_k^T)^T @ outT -> per-b logits
matmuls [n,b] -> PE transpose -> tanh/mask -> argmax via DVE max/max_index
-> mask/dyn/logp bookkeeping -> next state via indirect-DMA row gather.
All ln() calls deferred to one batched ACT call after the loop (keeps the
ACT table set fixed at exp/tanh in-loop); logp = -sum_t ln(W_t).
"""

import os
import sys
import threading
from contextlib import ExitStack

import numpy as np

for _p in ("/opt/trn_rl_repo", "/root/.axon_site/_ro/trn_rl_repo"):
    if os.path.isdir(_p) and _p not in sys.path:
        sys.path.insert(0, _p)

import concourse.bass as bass
import concourse.mybir as mybir
from concourse import bacc
from concourse.tile import TileContext

F32 = mybir.dt.float32
I32 = mybir.dt.int32
U32 = mybir.dt.uint32
OP = mybir.AluOpType
AF = mybir.ActivationFunctionType

B = 128      # batch rows per core
N = 101      # nodes
D = 128      # model dim
H = 8        # heads
HD = D // H
INV_HD = float(1.0 / np.sqrt(HD))
INV_D = float(1.0 / np.sqrt(D))
N_CORES = 8

_cache = {}
_cache_lock = threading.Lock()


def _build(n_steps: int, unrolled: bool = False):
    nc = bacc.Bacc("TRN2", target_bir_lowering=False, debug=False,
                   num_devices=N_CORES)

    enc_rows = nc.dram_tensor("enc_rows", [B * N, D], F32, kind="ExternalInput")
    demand_d = nc.dram_tensor("demand", [B, N], F32, kind="ExternalInput")
    cap_d = nc.dram_tensor("capcity", [B, 1], F32, kind="ExternalInput")
    capfull_d = nc.dram_tensor("capfull", [B, 1], F32, kind="ExternalInput")
    pool_d = nc.dram_tensor("pool", [B, D], F32, kind="ExternalInput")
    fcw_main_d = nc.dram_tensor("fc_w_main", [D, D], F32, kind="ExternalInput")
    fcw_last_d = nc.dram_tensor("fc_w_last", [1, D], F32, kind="ExternalInput")
    fc1_d = nc.dram_tensor("fc1_w", [D, D], F32, kind="ExternalInput")
    attn_w_d = nc.dram_tensor("attn_w", [D, D], F32, kind="ExternalInput")
    attn_k_d = nc.dram_tensor("attn_k", [D, D], F32, kind="ExternalInput")
    attn_v_d = nc.dram_tensor("attn_v", [D, D], F32, kind="ExternalInput")
    c2_d = nc.dram_tensor("c2", [D, D], F32, kind="ExternalInput")
    rconst_d = nc.dram_tensor("r_const", [H, B], F32, kind="ExternalInput")
    m8_d = nc.dram_tensor("m8", [B, H], F32, kind="ExternalInput")
    ident_d = nc.dram_tensor("ident_in", [B, B], F32, kind="ExternalInput")
    iota_d = nc.dram_tensor("iota_in", [B, N], F32, kind="ExternalInput")
    rowbase_d = nc.dram_tensor("rowbase_in", [B, 1], I32, kind="ExternalInput")

    actions_d = nc.dram_tensor("actions", [B, n_steps], I32, kind="ExternalOutput")
    logps_d = nc.dram_tensor("logps", [B, 1], F32, kind="ExternalOutput")

    with TileContext(nc) as tc, ExitStack() as ctx:
        res = ctx.enter_context(tc.tile_pool(name="res", bufs=1))
        stage = ctx.enter_context(tc.tile_pool(name="stage", bufs=3))
        pp = ctx.enter_context(tc.tile_pool(name="pp", bufs=1, space="PSUM"))
        pshared = ctx.enter_context(tc.tile_pool(name="pshared", bufs=2, space="PSUM"))

        encT = res.tile([D, B * N], F32)
        kT = res.tile([D, B * N], F32)
        v_nT = res.tile([B, B * D], F32)
        e_sb = res.tile([B, B * H], F32)
        q_bd = res.tile([D, B * H], F32)
        negI = res.tile([B, B], F32)
        ident = res.tile([B, B], F32)
        fcw_main = res.tile([D, D], F32)
        fcw_last = res.tile([1, D], F32)
        attn_w = res.tile([D, D], F32)
        c2 = res.tile([D, D], F32)
        pooledT = res.tile([D, B], F32)
        demand = res.tile([B, N], F32)
        mask = res.tile([B, N], F32)
        mask1 = res.tile([B, N], F32)
        neg9mask = res.tile([B, N], F32)
        onehot = res.tile([B, N], F32)
        dgt = res.tile([B, N], F32)
        mx = res.tile([B, N], F32)
        scr101 = res.tile([B, N], F32)
        iota_nf = res.tile([B, N], F32)
        rowbase_i = res.tile([B, 1], I32)
        dyn = res.tile([B, 1], F32)
        capf = res.tile([B, 1], F32)
        onesc = res.tile([B, 1], F32)
        R_const = res.tile([H, B], F32)
        M8 = res.tile([B, H], F32)
        stateT = res.tile([D, B], F32)
        state_g = res.tile([B, D], F32)
        dec_inT = res.tile([D, B], F32)
        outT_u = res.tile([D, B], F32)
        outT_s = res.tile([D, B], F32)
        u_sb = res.tile([D, B], F32)
        lt_nb = res.tile([B, B], F32)
        ltanh = res.tile([B, N], F32)
        logits_sb = res.tile([B, N], F32)
        Sinv_sb = res.tile([B, H], F32)
        SinvT_sb = res.tile([H, B], F32)
        max8 = res.tile([B, 8], F32)
        idx8u = res.tile([B, 8], U32)
        idx_f = res.tile([B, 1], F32)
        idx_i = res.tile([B, 1], I32)
        gidx_i = res.tile([B, 1], I32)
        neg_lmax = res.tile([B, 1], F32)
        sume = res.tile([B, 1], F32)
        sm1_old = res.tile([B, 1], F32)
        nd_old = res.tile([B, 1], F32)
        sm1_new = res.tile([B, 1], F32)
        nd_new = res.tile([B, 1], F32)
        sd = res.tile([B, 1], F32)
        gd = res.tile([B, 1], F32)
        gd_i = res.tile([B, 1], I32)
        nd_old_i = res.tile([B, 1], I32)
        dmg = res.tile([B, 1], F32)
        dep = res.tile([B, 1], F32)
        Wsel = res.tile([B, 1], F32)
        W_buf = res.tile([B, n_steps], F32)
        lnW = res.tile([B, n_steps], F32)
        sumln = res.tile([B, 1], F32)
        logp_out = res.tile([B, 1], F32)
        act_buf = res.tile([B, n_steps], I32)
        ps_dyT_sb = res.tile([1, B], F32)

        psc = pp.tile([B, B * H], F32)
        pso = pp.tile([B, B * H], F32)
        ps_rep = pp.tile([D, B], F32)
        ps_dyT = pp.tile([1, B], F32)

        # ============ INIT ============
        nc.gpsimd.dma_start(fcw_main[:], fcw_main_d[:, :])
        nc.gpsimd.dma_start(fcw_last[:], fcw_last_d[:, :])
        nc.gpsimd.dma_start(attn_w[:], attn_w_d[:, :])
        nc.gpsimd.dma_start(c2[:], c2_d[:, :])
        nc.gpsimd.dma_start(demand[:], demand_d[:, :])
        nc.gpsimd.dma_start(dyn[:], cap_d[:, :])
        nc.gpsimd.dma_start(capf[:], capfull_d[:, :])
        attn_k_sb = stage.tile([D, D], F32, tag="wtmp")
        attn_v_sb = stage.tile([D, D], F32, tag="wtmp")
        fc1_sb = stage.tile([D, D], F32, tag="wtmp")
        nc.gpsimd.dma_start(attn_k_sb[:], attn_k_d[:, :])
        nc.gpsimd.dma_start(attn_v_sb[:], attn_v_d[:, :])
        nc.gpsimd.dma_start(fc1_sb[:], fc1_d[:, :])
        nc.gpsimd.dma_start(R_const[:], rconst_d[:, :])
        nc.gpsimd.dma_start(M8[:], m8_d[:, :])
        nc.gpsimd.dma_start(ident[:], ident_d[:, :])
        nc.gpsimd.dma_start(iota_nf[:], iota_d[:, :])
        nc.gpsimd.dma_start(rowbase_i[:], rowbase_d[:, :])
        nc.vector.tensor_scalar_mul(negI[:], ident[:], -1e9)
        nc.vector.memset(onesc[:], 1.0)
        nc.vector.memset(q_bd[:], 0.0)
        nc.vector.memset(lt_nb[:], 0.0)

        # encT via per-n transposes
        for n in range(N):
            st_in = stage.tile([B, D], F32, tag="encstage")
            nc.sync.dma_start(st_in[:], enc_rows[n::N, :])
            ps_t = pshared.tile([B, D], F32, tag="ps_sh")
            nc.tensor.transpose(ps_t[:], st_in[:], ident[:])
            if n % 2 == 0:
                nc.vector.tensor_copy(encT[:, n::N], ps_t[:])
            else:
                nc.scalar.copy(encT[:, n::N], ps_t[:])

        # kT = attn_k^T @ encT
        CH = 512
        for c in range(0, B * N, CH):
            w = min(CH, B * N - c)
            ps_k = pshared.tile([D, CH], F32, tag="ps_sh")
            nc.tensor.matmul(ps_k[:, :w], lhsT=attn_k_sb[:], rhs=encT[:, c:c + w],
                             start=True, stop=True)
            if (c // CH) % 2 == 0:
                nc.vector.tensor_copy(kT[:, c:c + w], ps_k[:, :w])
            else:
                nc.scalar.copy(kT[:, c:c + w], ps_k[:, :w])

        # v_nT per-b
        for b in range(B):
            ps_v = pshared.tile([B, D], F32, tag="ps_sh")
            nc.tensor.matmul(ps_v[:N, :], lhsT=encT[:, b * N:(b + 1) * N],
                             rhs=attn_v_sb[:], start=True, stop=True)
            if b % 2 == 0:
                nc.vector.tensor_copy(v_nT[:N, b * D:(b + 1) * D], ps_v[:N, :])
            else:
                nc.scalar.copy(v_nT[:N, b * D:(b + 1) * D], ps_v[:N, :])

        # pooledT
        pool_sb = stage.tile([B, D], F32, tag="encstage")
        nc.gpsimd.dma_start(pool_sb[:], pool_d[:, :])
        ps_pt = pshared.tile([D, B], F32, tag="ps_sh")
        nc.tensor.transpose(ps_pt[:], pool_sb[:], ident[:])
        poolT_sb = stage.tile([D, B], F32, tag="encstage")
        nc.vector.tensor_copy(poolT_sb[:], ps_pt[:])
        ps_pl = pshared.tile([D, B], F32, tag="ps_sh")
        nc.tensor.matmul(ps_pl[:], lhsT=fc1_sb[:], rhs=poolT_sb[:], start=True,
                         stop=True)
        nc.vector.tensor_copy(pooledT[:], ps_pl[:])

        # state0 = enc[:, 0, :]; dynT0; mask init
        nc.sync.dma_start(state_g[:], enc_rows[0::N, :])
        ps_s0 = pshared.tile([D, B], F32, tag="ps_sh")
        nc.tensor.transpose(ps_s0[:], state_g[:], ident[:])
        nc.vector.tensor_copy(stateT[:], ps_s0[:])
        nc.tensor.transpose(ps_dyT[:], dyn[:], ident[:])
        nc.vector.memset(mask1[:], 0.0)
        nc.vector.tensor_scalar(mask[:], demand[:], dyn[:, 0:1], None, op0=OP.is_gt)
        nc.vector.memset(mask[:, 0:1], 1.0)
        nc.vector.tensor_scalar_mul(neg9mask[:], mask[:], -1e9)
        nc.vector.tensor_copy(ps_dyT_sb[:], ps_dyT[:])

        # ============ STEP ============
        def emit_step(t):
            ps_dec = pshared.tile([D, B], F32, tag="ps_sh")
            nc.tensor.matmul(ps_dec[:], lhsT=fcw_main[:], rhs=stateT[:],
                             start=True, stop=False)
            nc.tensor.matmul(ps_dec[:], lhsT=fcw_last[:], rhs=ps_dyT_sb[:],
                             start=False, stop=True)
            nc.vector.scalar_tensor_tensor(out=dec_inT[:], in0=ps_dec[:], scalar=1.0,
                                           in1=pooledT[:], op0=OP.mult, op1=OP.add)
            ps_q = pshared.tile([D, B], F32, tag="ps_sh")
            nc.tensor.matmul(ps_q[:], lhsT=attn_w[:], rhs=dec_inT[:], start=True,
                             stop=True)
            for half in range(2):
                bs = slice(half * 64, (half + 1) * 64)
                nc.vector.tensor_tensor(
                    q_bd[:].rearrange("p (b h) -> p b h", h=H)[:, bs, :],
                    ps_q[:, bs].to_broadcast([D, 64, H]),
                    M8[:].rearrange("p (x h) -> p x h", x=1).to_broadcast([D, 64, H]),
                    op=OP.mult)

            for c in range(2):
                for b in range(c * 64, (c + 1) * 64):
                    nc.tensor.matmul(psc[:N, b * H:(b + 1) * H],
                                     lhsT=kT[:, b * N:(b + 1) * N],
                                     rhs=q_bd[:, b * H:(b + 1) * H],
                                     start=(b % 64 == 0), stop=False,
                                     skip_group_check=True)
                rhs_ap = negI[:, c * 64:(c + 1) * 64].to_broadcast([B, 64, H])
                nc.tensor.matmul(psc[:N, c * 512:(c + 1) * 512],
                                 lhsT=mask[:], rhs=rhs_ap,
                                 start=False, stop=True, skip_group_check=True)
                nc.scalar.activation(e_sb[:N, c * 512:(c + 1) * 512],
                                     psc[:N, c * 512:(c + 1) * 512],
                                     AF.Exp, scale=INV_HD)
            for b in range(B):
                nc.tensor.matmul(pso[:, b * H:(b + 1) * H],
                                 lhsT=v_nT[:N, b * D:(b + 1) * D],
                                 rhs=e_sb[:N, b * H:(b + 1) * H],
                                 start=(b % 64 == 0), stop=(b % 64 == 63),
                                 skip_group_check=True)
            e_h = e_sb[:N, :].rearrange("p (b h) -> p h b", h=H)
            psS = pshared.tile([B, H], F32, tag="ps_sh")
            for h in range(H):
                nc.tensor.matmul(psS[:, h:h + 1], lhsT=e_h[:, h, :], rhs=onesc[:N, :],
                                 start=(h == 0), stop=(h == H - 1),
                                 skip_group_check=True)
            nc.vector.reciprocal(Sinv_sb[:], psS[:])
            ps_sT = pshared.tile([H, B], F32, tag="ps_sh")
            nc.tensor.transpose(ps_sT[:], Sinv_sb[:], ident[:])
            nc.vector.tensor_copy(SinvT_sb[:], ps_sT[:])
            nc.tensor.matmul(ps_rep[:], lhsT=R_const[:], rhs=SinvT_sb[:],
                             start=True, stop=True)

            nc.vector.tensor_tensor(
                q_bd[:].rearrange("p (b h) -> p b h", h=H),
                pso[:].rearrange("p (b h) -> p b h", h=H),
                M8[:].rearrange("p (x h) -> p x h", x=1).to_broadcast([D, B, H]),
                op=OP.mult)
            nc.vector.reduce_sum(outT_u[:],
                                 q_bd[:].rearrange("p (b h) -> p b h", h=H),
                                 axis=mybir.AxisListType.X)
            nc.vector.tensor_tensor(outT_s[:], outT_u[:], ps_rep[:], op=OP.mult)

            ps_u = pshared.tile([D, B], F32, tag="ps_sh")
            nc.tensor.matmul(ps_u[:], lhsT=c2[:], rhs=outT_s[:], start=True, stop=True)
            nc.vector.tensor_copy(u_sb[:], ps_u[:])
            ps_lg = pshared.tile([B, B], F32, tag="ps_sh")
            for b in range(B):
                nc.tensor.matmul(ps_lg[:N, b:b + 1],
                                 lhsT=encT[:, b * N:(b + 1) * N],
                                 rhs=u_sb[:, b:b + 1],
                                 start=(b == 0), stop=(b == B - 1),
                                 skip_group_check=True)
            nc.vector.tensor_copy(lt_nb[:N, :], ps_lg[:N, :])
            ps_lt = pshared.tile([B, B], F32, tag="ps_sh")
            nc.tensor.transpose(ps_lt[:], lt_nb[:], ident[:])
            nc.scalar.activation(ltanh[:], ps_lt[:, :N], AF.Tanh, scale=INV_D)
            nc.vector.scalar_tensor_tensor(out=logits_sb[:], in0=ltanh[:], scalar=10.0,
                                           in1=neg9mask[:], op0=OP.mult, op1=OP.add)

            nc.vector.max(out=max8[:], in_=logits_sb[:])
            nc.vector.max_index(out=idx8u[:], in_max=max8[:], in_values=logits_sb[:])
            nc.vector.tensor_copy(idx_f[:], idx8u[:, 0:1])
            nc.vector.tensor_copy(idx_i[:], idx8u[:, 0:1])
            nc.vector.tensor_tensor(gidx_i[:], idx_i[:], rowbase_i[:], op=OP.add)
            nc.gpsimd.indirect_dma_start(
                out=state_g[:], out_offset=None, in_=enc_rows[:, :],
                in_offset=bass.IndirectOffsetOnAxis(ap=gidx_i[:, 0:1], axis=0))
            ps_st = pshared.tile([D, B], F32, tag="ps_sh")
            nc.tensor.transpose(ps_st[:], state_g[:], ident[:])
            nc.vector.tensor_copy(stateT[:], ps_st[:])
            nc.vector.tensor_scalar(onehot[:], iota_nf[:], idx_f[:, 0:1], None,
                                    op0=OP.is_equal)
            nc.vector.tensor_scalar_mul(neg_lmax[:], max8[:, 0:1], -1.0)
            nc.scalar.activation(scr101[:], logits_sb[:], AF.Exp,
                                 bias=neg_lmax[:, 0:1], accum_out=sume[:])
            nc.vector.reduce_sum(sm1_old[:], mask1[:, 1:N], axis=mybir.AxisListType.X)
            nc.vector.tensor_scalar(nd_old[:], sm1_old[:], float(N - 1), None,
                                    op0=OP.is_lt)
            nc.vector.tensor_scalar(nd_old_i[:], sm1_old[:], float(N - 1), None,
                                    op0=OP.is_lt)
            nc.vector.select(Wsel[:], nd_old_i[:], sume[:], onesc[:])
            nc.vector.tensor_copy(W_buf[:, bass.ds(t, 1)], Wsel[:])
            nc.vector.tensor_copy(act_buf[:, bass.ds(t, 1)], idx_i[:])
            nc.vector.tensor_tensor(scr101[:], onehot[:], demand[:], op=OP.mult)
            nc.vector.reduce_sum(sd[:], scr101[:], axis=mybir.AxisListType.X)
            nc.vector.tensor_scalar(gd[:], idx_f[:], 0.5, None, op0=OP.is_lt)
            nc.vector.tensor_scalar(gd_i[:], idx_f[:], 0.5, None, op0=OP.is_lt)
            nc.vector.tensor_tensor(dmg[:], dyn[:], sd[:], op=OP.subtract)
            nc.vector.select(dyn[:], gd_i[:], capf[:], dmg[:])
            nc.vector.tensor_tensor(mx[:], mask1[:], onehot[:], op=OP.max)
            nc.vector.select(scr101[:], gd_i[:, 0:1].to_broadcast([B, N]), mask1[:],
                             mx[:])
            nc.vector.tensor_copy(mask1[:], scr101[:])
            nc.vector.reduce_sum(sm1_new[:], mask1[:, 1:N], axis=mybir.AxisListType.X)
            nc.vector.tensor_scalar(nd_new[:], sm1_new[:], float(N - 1) - 0.5, None,
                                    op0=OP.is_le)
            nc.vector.tensor_scalar(dgt[:], demand[:], dyn[:, 0:1], None, op0=OP.is_gt)
            nc.vector.tensor_tensor(mask[:], mask1[:], dgt[:], op=OP.max)
            nc.vector.tensor_tensor(dep[:], gd[:], nd_new[:], op=OP.mult)
            nc.vector.tensor_copy(mask[:, 0:1], dep[:])
            nc.vector.tensor_scalar_mul(neg9mask[:], mask[:], -1e9)

            nc.tensor.transpose(ps_dyT[:], dyn[:], ident[:])
            nc.vector.tensor_copy(ps_dyT_sb[:], ps_dyT[:])

        if unrolled:
            for t in range(n_steps):
                emit_step(t)
        else:
            with tc.For_i(0, n_steps, 1,
                          hint_engines=(mybir.EngineType.PE,)) as t:
                emit_step(t)

        # ============ TAIL ============
        nc.scalar.activation(lnW[:], W_buf[:], AF.Ln)
        nc.vector.reduce_sum(sumln[:], lnW[:], axis=mybir.AxisListType.X)
        nc.vector.tensor_scalar_mul(logp_out[:], sumln[:], -1.0)
        nc.gpsimd.dma_start(logps_d[:, :], logp_out[:])
        nc.gpsimd.dma_start(actions_d[:, :], act_buf[:])

    nc.compile()
    return nc


def _get_program(n_steps: int):
    with _cache_lock:
        if n_steps not in _cache:
            _cache[n_steps] = _build(n_steps)
        return _cache[n_steps]


def _make_in_maps(inputs):
    enc = np.ascontiguousarray(np.asarray(inputs["encoder_inputs"], np.float32))
    pool = np.asarray(inputs["pool"], np.float32)
    cap = np.asarray(inputs["capcity"], np.float32)
    dem = np.asarray(inputs["demand"], np.float32)
    fc_w = np.asarray(inputs["fc_w"], np.float32)
    fc1_w = np.ascontiguousarray(np.asarray(inputs["fc1_w"], np.float32))
    attn_w = np.ascontiguousarray(np.asarray(inputs["attn_w"], np.float32))
    attn_k = np.ascontiguousarray(np.asarray(inputs["attn_k"], np.float32))
    attn_v = np.ascontiguousarray(np.asarray(inputs["attn_v"], np.float32))
    attn_fc = np.asarray(inputs["attn_fc"], np.float32)
    prob_k = np.asarray(inputs["prob_k"], np.float32)
    c2 = np.ascontiguousarray(attn_fc @ prob_k.T)
    cap_full = np.full((B, 1), cap[0, 0], np.float32)
    shared = {
        "fc_w_main": np.ascontiguousarray(fc_w[:D]),
        "fc_w_last": np.ascontiguousarray(fc_w[D:D + 1]),
        "fc1_w": fc1_w, "attn_w": attn_w, "attn_k": attn_k,
        "attn_v": attn_v, "c2": c2, "capfull": cap_full,
        "r_const": np.repeat(np.eye(H, dtype=np.float32), HD, axis=1),
        "m8": np.repeat(np.eye(H, dtype=np.float32), HD, axis=0),
        "ident_in": np.eye(B, dtype=np.float32),
        "iota_in": np.tile(np.arange(N, dtype=np.float32), (B, 1)),
        "rowbase_in": (np.arange(B, dtype=np.int32) * N)[:, None],
    }
    maps = []
    for c in range(N_CORES):
        s = slice(c * B, (c + 1) * B)
        m = dict(shared)
        m["enc_rows"] = np.ascontiguousarray(enc[s].reshape(B * N, D))
        m["demand"] = np.ascontiguousarray(dem[s])
        m["capcity"] = np.ascontiguousarray(cap[s])
        m["pool"] = np.ascontiguousarray(pool[s])
        maps.append(m)
    return maps


def kernel(**inputs):
    n_steps = int(np.asarray(inputs.get("n_steps", 142)))
    assert int(np.asarray(inputs.get("num_depots", 1))) == 1
    assert int(np.asarray(inputs.get("temperature", 1))) == 1
    assert int(np.asarray(inputs.get("greedy", 1))) == 1
    Btot = np.asarray(inputs["encoder_inputs"]).shape[0]
    assert Btot == B * N_CORES

    from concourse.bass_utils import run_bass_kernel_spmd

    nc = _get_program(n_steps)
    maps = _make_in_maps(inputs)
    res = run_bass_kernel_spmd(nc, maps, core_ids=list(range(N_CORES)))
    actions = np.concatenate(
        [res.results[c]["actions"] for c in range(N_CORES)], axis=0).astype(np.int32)
    logps = np.concatenate(
        [res.results[c]["logps"][:, 0] for c in range(N_CORES)], axis=0
    ).astype(np.float32)
    return actions, logps
